# revision 25
# baseline (speedup 1.0000x reference)
"""BinaryConv2D Trainium2 kernel — 1D Winograd F(2,3) along W.

Full computation:
  out = conv2d(sign(pad(x)), sign(k)) * avgpool3x3(mean|pad(x)|_ci) * alpha + bias

Device strategy (8 NeuronCores, data-parallel over batch N=32 -> 4 images/core):
  - Host binarizes x/k and applies the Winograd F(2,3) input transform along W:
    for each output column pair (2t, 2t+1) the 4-tap window d = xpad[2t:2t+4]
    becomes V0 = d0-d2, V1 = d1+d2, V2 = d2-d1, V3 = d1-d3 (values in {-2,0,2},
    exact in fp8e4m3).  Weights get U0 = g0, U1 = (g0+g1+g2)/2,
    U2 = (g0-g1+g2)/2, U3 = g2 per kh row (exact half-integers in fp8).
  - Device: for each (img, co-half, row group) accumulate P_j = sum_kh
    U[j,kh]^T @ V_j[rows+kh] into PSUM (3 matmuls each, fp8 DoubleRow
    contracting ci=256).  12 matmuls per group vs 18 direct-conv
    equivalents: 1.5x fewer PE cycles.  Row groups are (16,16,16,8) so the
    matmul free size (448/224) is a multiple of 16 (DoubleRow requirement)
    and a P-plane fits one PSUM bank.
  - Epilogue (Y0 = P0+P1+P2 -> even cols, Y1 = P1-P2-P3 -> odd cols) is split
    across engines around three constraints: GPSIMD cannot touch PSUM, a
    tensor_tensor may read at most ONE operand from PSUM, and GPSIMD is slow
    (~2.1ns/elem):
      Scalar: c1 = copy(P1)
      DVE:    t0 = c1+P0, t1 = c1-P2, y0 = t0+P2, y1 = t1-P3
      Pool:   z0 = y0*Ka[even], z1 = y1*Ka[odd]   (bf16 out, parity-planar)
    Ka = avgpool(mean|x|) * alpha (per co-half) is computed host-side in bf16;
    bias is added on the host after the gather.  Input DMAs are staged (early
    needs first) and never block the Sync queue; output is bf16.
"""

import sys

for _p in ("/root/.axon_site/_ro/trn_rl_repo", "/opt/trn_rl_repo"):
    if _p not in sys.path:
        sys.path.append(_p)

import numpy as np
import ml_dtypes

import concourse.bass as bass  # noqa: F401  (registers arch tables)
import concourse.mybir as mybir
import concourse.tile as tile
from concourse import bacc
from concourse.bass_utils import run_bass_kernel_spmd

FP8 = mybir.dt.float8e4
F32 = mybir.dt.float32
BF16 = mybir.dt.bfloat16
DR = mybir.MatmulPerfMode.DoubleRow
ADD = mybir.AluOpType.add
SUB = mybir.AluOpType.subtract
MUL = mybir.AluOpType.mult

NCORES = 8
N, H, W, C = 32, 56, 56, 256
HP = H + 2                      # padded rows
T = W // 2                      # 28 Winograd tiles along W
VCOLS = HP * T                  # 1624 = flat (row, tile) V-plane size
VFREE = 1632                    # padded V free size
GROUPS = [(0, 16), (16, 16), (32, 12), (44, 12)]  # (row0, nrows) per group
OPAIRS = H * T                  # 1568 output pairs per (img, co-half)
V0COLS = 18 * T                 # 504: V cols needed by group 0 (16+2 rows)
V0FREE = 512
VSPLIT = 34 * T                 # 952: V cols for groups 0-1 (img0 split DMA)
NIMG = N // NCORES

_NC = None


def _build_nc():
    nc = bacc.Bacc("TRN2", target_bir_lowering=False, debug=False)

    ub = nc.dram_tensor("ub", [128, 4, 3, 2, C], FP8, kind="ExternalInput")
    vb = nc.dram_tensor("vb", [NIMG, 4, 2, 128, VFREE], FP8, kind="ExternalInput")
    kb = nc.dram_tensor("kb", [NIMG, 2, 2, 128, OPAIRS], BF16, kind="ExternalInput")
    ob = nc.dram_tensor(
        "ob", [NIMG, 2, 128, 2, OPAIRS], BF16, kind="ExternalOutput"
    )

    with tile.TileContext(nc) as tc:
        with (
            tc.tile_pool(name="wp", bufs=1) as wp,
            tc.tile_pool(name="xp", bufs=4) as xp,
            tc.tile_pool(name="kp", bufs=4) as kp,
            tc.tile_pool(name="sc", bufs=4) as sc,
            tc.tile_pool(name="op", bufs=6) as op,
            tc.tile_pool(name="ps", bufs=8, space="PSUM") as ps,
        ):
            # allocate all input tiles up front (bufs deep enough that nothing
            # ever waits); DMAs are issued in need-order
            w_sb = wp.tile([128, 4, 3, 2, C], FP8, tag="w")
            v_tiles = [
                xp.tile([128, 4, 2, VFREE], FP8, tag="v", name="v")
                for _ in range(NIMG)
            ]
            k_tiles = [
                kp.tile([128, 2, 2, OPAIRS], BF16, tag="k", name="k")
                for _ in range(NIMG)
            ]

            # startup DMAs staggered per j-plane so the j=0 matmuls' exact
            # dependencies (smallest possible transfers) land first; vb0 comes
            # in three column-range pieces (subtile deps let group g wait only
            # for the piece it reads)
            for j in range(4):
                nc.sync.dma_start(w_sb[:, j : j + 1], ub[:, j : j + 1])
                nc.sync.dma_start(
                    v_tiles[0][:, j, :, 0:V0COLS],
                    vb[0, j, :, :, 0:V0COLS].rearrange("k p f -> p k f"),
                )

            # tiny warmup matmuls start the PE p-state ramp while DMAs fly;
            # they recycle one slot of the PSUM ring
            scr = wp.tile([128, 2, 160], FP8, tag="scr")
            nc.vector.memset(scr[:], 0)
            warm = ps.tile([128, 448], F32, tag="pt", bufs=4)
            for _ in range(22):
                nc.tensor.matmul(
                    warm[:, 0:16], scr[:, :, 0:128], scr[:, :, 0:16],
                    start=True, stop=True, perf_mode=DR,
                )

            # img0 V rest: groups 1 then 2-3
            nc.sync.dma_start(
                v_tiles[0][:, :, :, V0COLS:VSPLIT],
                vb[0, :, :, :, V0COLS:VSPLIT].rearrange("j k p f -> p j k f"),
            )
            nc.sync.dma_start(
                v_tiles[0][:, :, :, VSPLIT:VFREE],
                vb[0, :, :, :, VSPLIT:VFREE].rearrange("j k p f -> p j k f"),
            )
            nc.sync.dma_start(
                k_tiles[0][:, 0], kb[0, 0].rearrange("i p f -> p i f")
            )
            nc.sync.dma_start(
                k_tiles[0][:, 1], kb[0, 1].rearrange("i p f -> p i f")
            )

            for img in range(NIMG):
                v_sb = v_tiles[img]
                k_sb = k_tiles[img]
                for c in range(2):
                    if c == 1 and img + 1 < NIMG:
                        # next image's inputs: ~11us lead, no competition with
                        # this image's own transfers (already landed)
                        nc.sync.dma_start(
                            v_tiles[img + 1][:],
                            vb[img + 1].rearrange("j k p f -> p j k f"),
                        )
                        nc.sync.dma_start(
                            k_tiles[img + 1][:, 0],
                            kb[img + 1, 0].rearrange("i p f -> p i f"),
                        )
                        nc.sync.dma_start(
                            k_tiles[img + 1][:, 1],
                            kb[img + 1, 1].rearrange("i p f -> p i f"),
                        )
                    for row0, nrows in GROUPS:
                        gc = nrows * T
                        goff = row0 * T
                        # j0/j1 in single-bank tiles; j2/j3 share one 2-bank
                        # tile (U3 is host-negated) so y0 = t0+P2 and
                        # y1 = t1+(-P3) merge into ONE 896-wide DVE op reading
                        # a bank-spanning PSUM operand, and both K-multiplies
                        # merge into ONE GpSimd op
                        pts = []
                        ptd = None
                        for j in range(4):
                            if j < 2:
                                pt = ps.tile([128, 448], F32, tag="pt", bufs=4)
                                dst = pt[:, 0:gc]
                                pts.append(pt)
                            else:
                                if ptd is None:
                                    ptd = ps.tile(
                                        [128, 2, 512], F32, tag="ptd",
                                        name="ptd", bufs=2,
                                    )
                                dst = ptd[:, j - 2, 0:gc]
                            for kh in range(3):
                                off = (row0 + kh) * T
                                src = v_sb[:, j, :, off : off + gc]
                                nc.tensor.matmul(
                                    dst,
                                    w_sb[:, j, kh, :, c * 128 : (c + 1) * 128],
                                    src,
                                    start=(kh == 0),
                                    stop=(kh == 2),
                                    perf_mode=DR,
                                )
                        p0, p1 = pts[0][:, 0:gc], pts[1][:, 0:gc]
                        p2 = ptd[:, 0, 0:gc]

                        c1 = sc.tile([128, 448], F32, tag="c1", name="c1")[:, 0:gc]
                        nc.scalar.copy(c1, p1)
                        tt = sc.tile([128, 2, 448], F32, tag="tt", name="tt")
                        nc.vector.tensor_tensor(tt[:, 0, 0:gc], c1, p0, ADD)
                        nc.vector.tensor_tensor(tt[:, 1, 0:gc], c1, p2, SUB)
                        yy = sc.tile([128, 2, 448], F32, tag="yy", name="yy")
                        nc.vector.tensor_tensor(
                            yy[:, :, 0:gc], tt[:, :, 0:gc], ptd[:, :, 0:gc], ADD
                        )

                        o_sb = op.tile([128, 2, 448], BF16, tag="o", name="o")[
                            :, :, 0:gc
                        ]
                        nc.gpsimd.tensor_tensor(
                            o_sb, yy[:, :, 0:gc], k_sb[:, c, :, goff : goff + gc], MUL
                        )
                        nc.sync.dma_start(
                            ob[img, c, :, :, goff : goff + gc],
                            o_sb,
                        )

    nc.compile()
    return nc


def get_nc():
    global _NC
    if _NC is None:
        _NC = _build_nc()
    return _NC


def prep_inputs(x, kernel, bias):
    """Host-side prep: binarize, pad, Winograd-transform; per-core in_maps."""
    np_fp8 = mybir.dt.np(FP8)
    xp_ = np.pad(x, ((0, 0), (1, 1), (1, 1), (0, 0)))
    binx = np.where(xp_ > 0, np.float32(1.0), np.float32(-1.0))
    bt = np.ascontiguousarray(binx.transpose(0, 3, 1, 2))  # (N, C, 58, 58)

    d0 = bt[:, :, :, 0:56:2]
    d1 = bt[:, :, :, 1:57:2]
    d2 = bt[:, :, :, 2:58:2]
    d3 = bt[:, :, :, 3:59:2]
    V = np.empty((N, 4, C, HP, T), np.float32)
    V[:, 0] = d0 - d2
    V[:, 1] = d1 + d2
    V[:, 2] = d2 - d1
    V[:, 3] = d1 - d3
    vb_all = np.zeros((N, 4, 2, 128, VFREE), np_fp8)
    vb_all[:, :, :, :, :VCOLS] = V.reshape(N, 4, 2, 128, VCOLS).astype(np_fp8)

    bink = np.where(kernel > 0, np.float32(1.0), np.float32(-1.0))
    g0, g1, g2 = bink[:, 0], bink[:, 1], bink[:, 2]  # (3, 256, 256) each: (kh, ci, co)
    U = np.empty((4, 3, C, C), np.float32)
    U[0] = g0
    U[1] = (g0 + g1 + g2) * np.float32(0.5)
    U[2] = (g0 - g1 + g2) * np.float32(0.5)
    U[3] = -g2  # negated: y1 = t1 + (-P3) keeps the merged y-op an ADD
    ub_host = np.ascontiguousarray(
        U.reshape(4, 3, 2, 128, C).transpose(3, 0, 1, 2, 4)
    ).astype(np_fp8)  # (128, 4, 3, 2, 256)

    alpha = np.abs(kernel).mean(axis=(0, 1, 2)).astype(np.float32)  # (256,)

    beta = np.abs(xp_).mean(axis=3)  # (N, 58, 58)
    ks = beta[:, 0:H, :] + beta[:, 1 : H + 1, :] + beta[:, 2 : H + 2, :]
    K = (ks[:, :, 0:W] + ks[:, :, 1 : W + 1] + ks[:, :, 2 : W + 2]) / np.float32(9.0)
    Kr = K.reshape(N, H, T, 2).transpose(0, 3, 1, 2).reshape(N, 1, 2, 1, OPAIRS)
    # Ka[n, c, i, p, f] = K[n, pixel(i, f)] * alpha[c*128 + p]
    Ka = Kr * alpha.reshape(1, 2, 1, 128, 1)
    kb_all = np.ascontiguousarray(Ka.astype(ml_dtypes.bfloat16))

    in_maps = []
    for core in range(NCORES):
        sl = slice(core * NIMG, (core + 1) * NIMG)
        in_maps.append(
            {
                "ub": ub_host,
                "vb": np.ascontiguousarray(vb_all[sl]),
                "kb": kb_all[sl],
            }
        )
    return in_maps


def assemble_output(results, bias):
    """results: 8 dicts with 'ob' (NIMG, 2, 128, 2, OPAIRS) bf16 -> NHWC f32."""
    ot = np.concatenate([r["ob"] for r in results], axis=0).astype(np.float32)
    o6 = ot.reshape(N, 2, 128, 2, H, T)  # (n, c, p, i, h, t)
    out = o6.transpose(0, 4, 5, 3, 1, 2).reshape(N, H, W, C)
    out += bias.astype(np.float32).reshape(1, 1, 1, C)
    return np.ascontiguousarray(out)


def kernel(x, kernel, bias, _trace=False):
    nc = get_nc()
    in_maps = prep_inputs(x, kernel, bias)
    res = run_bass_kernel_spmd(
        nc, in_maps, core_ids=list(range(NCORES)), trace=_trace
    )
    out = assemble_output(res.results, bias)
    if _trace:
        return out, res
    return out
